# revision 1
# baseline (speedup 1.0000x reference)
"""LucidLinearAttention Trainium2 kernel (8-core SPMD).

Sharding: batch b = core//2 (4 batches), head-group hg = core%2 (8 heads each).
Each core computes qkv projection for its heads, chunked linear attention
(bucket-exclusive cumsum) via a hybrid block-causal formulation, and its
partial output projection. Host sums the two head-group partials per batch.

All matmul accumulation groups use lhsT/rhs at partition base 0 with uniform
K (mixed-base accumulation groups crash the HW - validated by bisection).
"""
import sys
import numpy as np

for p in ("/opt/trn_rl_repo", "/root/.axon_site/_ro/trn_rl_repo"):
    if p not in sys.path:
        sys.path.insert(0, p)

import concourse.mybir as mybir
import concourse.tile as tile
from concourse import bacc
from concourse.bass_utils import run_bass_kernel_spmd
from concourse.masks import make_identity

F32 = mybir.dt.float32
F32R = mybir.dt.float32r
EXP = mybir.ActivationFunctionType.Exp

B, T, D = 4, 4096, 1024
NH, HD, BUCKET = 16, 64, 64
HPC = 8            # heads per core
GD = HPC * HD      # 512 group dim
NBLK = 8           # coarse blocks
BT = T // NBLK     # 512 rows per block
NC_CORES = 8

_CACHE = {}


def _build():
    nc = bacc.Bacc("TRN2", target_bir_lowering=False, debug=False,
                   num_devices=NC_CORES)
    xT = nc.dram_tensor("xT", [D, T], F32, kind="ExternalInput").ap()
    wqT = nc.dram_tensor("wqT", [D, GD], F32, kind="ExternalInput").ap()
    wkT = nc.dram_tensor("wkT", [D, GD], F32, kind="ExternalInput").ap()
    wvT = nc.dram_tensor("wvT", [D, GD], F32, kind="ExternalInput").ap()
    woT = nc.dram_tensor("woT", [GD, D], F32, kind="ExternalInput").ap()
    y = nc.dram_tensor("y", [T, D], F32, kind="ExternalOutput").ap()

    with tile.TileContext(nc) as tc:
        with nc.allow_low_precision(reason="float32r matmul rounding by design"), \
             tc.tile_pool(name="w", bufs=1) as wp, \
             tc.tile_pool(name="per", bufs=1) as pp, \
             tc.tile_pool(name="sb", bufs=1) as sbp, \
             tc.tile_pool(name="ps", bufs=1, space="PSUM") as ps:

            # ---- resident weights -------------------------------------
            wq_sb = [wp.tile([128, GD], F32R, tag=f"wq{dc}", name=f"wq{dc}") for dc in range(8)]
            wk_sb = [wp.tile([128, GD], F32R, tag=f"wk{dc}", name=f"wk{dc}") for dc in range(8)]
            wv_sb = [wp.tile([128, GD], F32R, tag=f"wv{dc}", name=f"wv{dc}") for dc in range(8)]
            wo_sb = [wp.tile([64, D], F32R, tag=f"wo{h}", name=f"wo{h}") for h in range(HPC)]
            for dc in range(8):
                for src_ap, dst in ((wqT, wq_sb), (wkT, wk_sb), (wvT, wv_sb)):
                    stg = sbp.tile([128, GD], F32, tag="stage", name="stage", bufs=2)
                    nc.sync.dma_start(stg[:], src_ap[128 * dc:128 * (dc + 1), :])
                    nc.vector.tensor_copy(dst[dc][:], stg[:])
            for h in range(HPC):
                stg = sbp.tile([64, D], F32, tag="wstage", name="wstage", bufs=2)
                nc.sync.dma_start(stg[:], woT[64 * h:64 * (h + 1), :])
                nc.vector.tensor_copy(wo_sb[h][:], stg[:])

            # ---- persistent state -------------------------------------
            ident = pp.tile([128, 128], F32, tag="ident")
            make_identity(nc, ident[:])
            ident_r = pp.tile([128, 128], F32R, tag="ident_r")
            nc.vector.tensor_copy(ident_r[:], ident[:])
            # F32 staging constants (memset on F32R is invalid ISA; fp32r
            # tiles must be produced by rounding compute instructions).
            zero_f32 = pp.tile([128, BT], F32, tag="zero_f32")
            nc.vector.memset(zero_f32[:], 0.0)
            one_f32 = pp.tile([128, 16], F32, tag="one_f32")
            nc.vector.memset(one_f32[:], 1.0)
            # bvec: K=2 broadcast weights; row 64 = 1, row 65 = 0.
            bv_f32 = pp.tile([66, 64], F32, tag="bv_f32")
            nc.vector.memset(bv_f32[:], 0.0)
            nc.vector.memset(bv_f32[64:65, :], 1.0)
            bvec = pp.tile([66, 64], F32R, tag="bvec")
            nc.vector.tensor_copy(bvec[:], bv_f32[:])
            # qtu_h: [128, BT]; rows 0-63 = exp(q) of head h (d x t),
            # rows 64-127 permanently zero (K=128 inter matmul padding).
            qtu = [pp.tile([128, BT], F32R, tag=f"qtu{h}", name=f"qtu{h}") for h in range(HPC)]
            for h in range(HPC):
                nc.vector.tensor_copy(qtu[h][:], zero_f32[:])
            # caug_h: [128, 66]; rows 0-63 = [C (d x e) | kcum | pad], rest 0.
            caug = [pp.tile([128, 66], F32R, tag=f"caug{h}", name=f"caug{h}") for h in range(HPC)]
            for h in range(HPC):
                nc.vector.tensor_copy(caug[h][:], zero_f32[:, 0:66])
            # vaug[tc]: [128, 8*66]; per head h cols h*66..h*66+64 = V,
            # col h*66+64 = ones (den trick), col h*66+65 = zero pad.
            vaug = [pp.tile([128, HPC * 66], F32R, tag=f"vaug{t}", name=f"vaug{t}") for t in range(4)]
            one_col = one_f32[:].rearrange("p (a b) -> p a b", b=1)[:, 0:8, :]
            zero_col = zero_f32[:, 0:8].rearrange("p (a b) -> p a b", b=1)
            for t4 in range(4):
                vv = vaug[t4][:].rearrange("p (h c) -> p h c", c=66)
                nc.vector.tensor_copy(vv[:, :, 64:65], one_col)
                nc.vector.tensor_copy(vv[:, :, 65:66], zero_col)
            # ssb: 2 parity sets x 4 chunks of masked S^T [128, BT].
            # Zero strips are preset once and never overwritten.
            ssb = [[pp.tile([128, BT], F32R, tag=f"ssb{s}_{t}", name=f"ssb{s}_{t}") for t in range(4)]
                   for s in range(2)]
            for s in range(2):
                for t4 in range(4):
                    nc.vector.tensor_copy(ssb[s][t4][:], zero_f32[:])

            # ---- main loop over coarse blocks -------------------------
            for ct in range(NBLK):
                t0 = ct * BT
                # x^T tiles for this block: [d-chunk 128, t 512]
                xsb = [sbp.tile([128, BT], F32R, tag=f"xsb{dc}", name=f"xsb{dc}") for dc in range(8)]
                for dc in range(8):
                    xstg = sbp.tile([128, BT], F32, tag="xstage", name="xstage", bufs=2)
                    nc.sync.dma_start(
                        xstg[:], xT[128 * dc:128 * (dc + 1), t0:t0 + BT])
                    nc.vector.tensor_copy(xsb[dc][:], xstg[:])

                # Q^T projection per head (M=64) + exp
                for h in range(HPC):
                    pq = ps.tile([64, BT], F32, tag="big")
                    for dc in range(8):
                        nc.tensor.matmul(
                            pq[:], wq_sb[dc][:, 64 * h:64 * (h + 1)], xsb[dc][:],
                            start=(dc == 0), stop=(dc == 7))
                    nc.scalar.activation(qtu[h][0:64, :], pq[:], EXP)

                # K natural projection per t-chunk (M=128) + exp
                ksb = [sbp.tile([128, GD], F32R, tag=f"ksb{t}", name=f"ksb{t}") for t in range(4)]
                for t4 in range(4):
                    pk = ps.tile([128, GD], F32, tag="big")
                    for dc in range(8):
                        nc.tensor.matmul(
                            pk[:], xsb[dc][:, 128 * t4:128 * (t4 + 1)], wk_sb[dc][:],
                            start=(dc == 0), stop=(dc == 7))
                    nc.scalar.activation(ksb[t4][:], pk[:], EXP)

                # V projection per t-chunk -> vaug strided cols
                for t4 in range(4):
                    pv = ps.tile([128, GD], F32, tag="big")
                    for dc in range(8):
                        nc.tensor.matmul(
                            pv[:], xsb[dc][:, 128 * t4:128 * (t4 + 1)], wv_sb[dc][:],
                            start=(dc == 0), stop=(dc == 7))
                    vv = vaug[t4][:].rearrange("p (h c) -> p h c", c=66)
                    pvv = pv[:].rearrange("p (h c) -> p h c", c=64)
                    nc.vector.tensor_copy(vv[:, :, 0:64], pvv[:, :, :])

                # K^T per head via PE transpose: kt_h [64, BT]
                kt = [sbp.tile([64, BT], F32R, tag=f"kt{h}", name=f"kt{h}") for h in range(HPC)]
                for h in range(HPC):
                    for t4 in range(4):
                        pt = ps.tile([64, 128], F32R, tag="small")
                        nc.tensor.transpose(
                            pt[:], ksb[t4][:, 64 * h:64 * (h + 1)], ident_r[:])
                        nc.vector.tensor_copy(
                            kt[h][:, 128 * t4:128 * (t4 + 1)], pt[:])

                # ---- attention per head -------------------------------
                xots = []
                for h in range(HPC):
                    par = h % 2
                    # S^T chunks + masked region copies
                    for t4 in range(4):
                        pst = ps.tile([128, BT], F32, tag="s")
                        nc.tensor.matmul(
                            pst[:], kt[h][:, 128 * t4:128 * (t4 + 1)],
                            qtu[h][0:64, :], start=True, stop=True)
                        c0 = (2 * t4 + 1) * 64
                        c1 = (2 * t4 + 2) * 64
                        nc.scalar.copy(ssb[par][t4][0:64, c0:BT], pst[0:64, c0:BT])
                        if c1 < BT:
                            nc.scalar.copy(
                                ssb[par][t4][64:128, c1:BT], pst[64:128, c1:BT])

                    # OUT group: inter (K=128, zero-padded) + 4 intra partial-N
                    po = ps.tile([66, BT], F32, tag="o")
                    nc.tensor.matmul(po[:], caug[h][:, :], qtu[h][:, :],
                                     start=True, stop=False)
                    for t4 in range(4):
                        n0 = (2 * t4 + 1) * 64
                        nc.tensor.matmul(
                            po[0:66, n0:BT],
                            vaug[t4][:, 66 * h:66 * h + 66],
                            ssb[par][t4][:, n0:BT],
                            start=False, stop=(t4 == 3))

                    # normalize: dinv = 1/max(den,eps); bcast via K=1 matmul
                    dv = sbp.tile([66, BT], F32R, tag="dv")
                    nc.vector.tensor_scalar_max(dv[64:66, :], po[64:66, :], 1e-30)
                    nc.vector.reciprocal(dv[64:66, :], dv[64:66, :])
                    pb = ps.tile([64, BT], F32, tag="small")
                    nc.tensor.matmul(pb[:], bvec[64:66, 0:64], dv[64:66, :],
                                     start=True, stop=True)
                    sbb = sbp.tile([64, BT], F32, tag="sbb")
                    nc.scalar.copy(sbb[:], pb[:])
                    xot = sbp.tile([64, BT], F32R, tag=f"xot{h}")
                    nc.vector.tensor_mul(xot[:], po[0:64, :], sbb[:])

                    # C/kcum update (after inter read): caug += K^T @ V_aug
                    pc = ps.tile([64, 66], F32, tag="small")
                    for t4 in range(4):
                        nc.tensor.matmul(
                            pc[:], ksb[t4][:, 64 * h:64 * (h + 1)],
                            vaug[t4][:, 66 * h:66 * h + 66],
                            start=(t4 == 0), stop=(t4 == 3))
                    nc.vector.tensor_add(caug[h][0:64, :], caug[h][0:64, :], pc[:])

                    xots.append(xot)

                # partial output projection + store y block
                for t4 in range(4):
                    for fc in range(2):
                        py = ps.tile([128, GD], F32, tag="big")
                        for h in range(HPC):
                            nc.tensor.matmul(
                                py[:],
                                xots[h][:, 128 * t4:128 * (t4 + 1)],
                                wo_sb[h][:, GD * fc:GD * (fc + 1)],
                                start=(h == 0), stop=(h == HPC - 1))
                        ysb = sbp.tile([128, GD], F32, tag="ysb")
                        nc.vector.tensor_copy(ysb[:], py[:])
                        nc.sync.dma_start(
                            y[t0 + 128 * t4:t0 + 128 * (t4 + 1),
                              GD * fc:GD * (fc + 1)], ysb[:])

    nc.compile()
    return nc


def _get_nc():
    if "nc" not in _CACHE:
        _CACHE["nc"] = _build()
    return _CACHE["nc"]


def kernel(x, W_qkv, W_out):
    x = np.asarray(x, dtype=np.float32)
    W_qkv = np.asarray(W_qkv, dtype=np.float32)
    W_out = np.asarray(W_out, dtype=np.float32)
    nc = _get_nc()

    xTs = [np.ascontiguousarray(x[b].T) for b in range(B)]
    in_maps = []
    for c in range(NC_CORES):
        b, hg = c // 2, c % 2
        s = slice(hg * GD, (hg + 1) * GD)
        in_maps.append({
            "xT": xTs[b],
            "wqT": np.ascontiguousarray(W_qkv[0 * D:1 * D][s].T),
            "wkT": np.ascontiguousarray(W_qkv[1 * D:2 * D][s].T),
            "wvT": np.ascontiguousarray(W_qkv[2 * D:3 * D][s].T),
            "woT": np.ascontiguousarray(W_out[:, s].T),
        })
    res = run_bass_kernel_spmd(nc, in_maps, core_ids=list(range(NC_CORES)))
    out = np.empty((B, T, D), dtype=np.float32)
    for b in range(B):
        out[b] = res.results[2 * b]["y"] + res.results[2 * b + 1]["y"]
    return out



# revision 3
# speedup vs baseline: 1.8870x; 1.8870x over previous
"""LucidLinearAttention Trainium2 kernel (8-core SPMD), v2.

Sharding: batch b = core//2 (4 batches), head-group hg = core%2 (8 heads each).
Each core: qkv projection for its heads, chunked linear attention over
BT=512 blocks with exact BUCKET=64 causal masking inside the block, partial
output projection. Host sums the two head-group partials per batch.

v2 over baseline:
- f32r DRAM tensors, DMA straight into f32r SBUF (no staging copies).
- bf16 for the attention inner loop (S^T, intra/inter context matmuls,
  per-chunk K transposes, C updates): both matmul inputs bf16 avoids the
  fp32r small-free-dim 4x penalty and halves SBUF.
- Q projection pair-packed (M=128) and Y projection pair-packed (K=128);
  odd heads cross from partition base 0 into the packed tiles' rows 64:127
  via SBUF->SBUF DMA (engine ops cannot change partition base).
- S^T matmuls restricted to the needed query range per key chunk.
- C/kcum state kept in f32 (caug_st), rounded to the bf16 matmul operand
  (caug_bf) each block so rounding does not compound.
- kcum initialized to 1e-30: den > 0 always, no clamp op needed.
- All 8 PSUM banks in use; hot pools double-buffered.
"""
import sys
import numpy as np

for p in ("/opt/trn_rl_repo", "/root/.axon_site/_ro/trn_rl_repo"):
    if p not in sys.path:
        sys.path.insert(0, p)

import concourse.mybir as mybir
import concourse.tile as tile
from concourse import bacc
from concourse.bass_utils import run_bass_kernel_spmd
from concourse.masks import make_identity

F32 = mybir.dt.float32
F32R = mybir.dt.float32r
BF16 = mybir.dt.bfloat16
EXP = mybir.ActivationFunctionType.Exp

B, T, D = 4, 4096, 1024
NH, HD, BUCKET = 16, 64, 64
HPC = 8            # heads per core
GD = HPC * HD      # 512 group dim
NBLK = 8           # coarse blocks
BT = T // NBLK     # 512 rows per block
NPAIR = 4
NC_CORES = 8

_CACHE = {}


def _build():
    nc = bacc.Bacc("TRN2", target_bir_lowering=False, debug=False,
                   num_devices=NC_CORES)
    xT = nc.dram_tensor("xT", [D, T], F32R, kind="ExternalInput").ap()
    wqT = nc.dram_tensor("wqT", [D, GD], F32R, kind="ExternalInput").ap()
    wkT = nc.dram_tensor("wkT", [D, GD], F32R, kind="ExternalInput").ap()
    wvT = nc.dram_tensor("wvT", [D, GD], F32R, kind="ExternalInput").ap()
    woT = nc.dram_tensor("woT", [GD, D], F32R, kind="ExternalInput").ap()
    y = nc.dram_tensor("y", [T, D], F32, kind="ExternalOutput").ap()

    with tile.TileContext(nc) as tc:
        with nc.allow_low_precision(reason="f32r/bf16 matmul rounding by design"), \
             tc.tile_pool(name="w", bufs=1) as wp, \
             tc.tile_pool(name="per", bufs=1) as pp, \
             tc.tile_pool(name="sb", bufs=1) as sbp, \
             tc.tile_pool(name="ps", bufs=1, space="PSUM") as ps:

            # ---- resident weights (DMA straight into f32r) --------------
            wq_sb = [wp.tile([128, GD], F32R, tag=f"wq{dc}", name=f"wq{dc}") for dc in range(8)]
            wk_sb = [wp.tile([128, GD], F32R, tag=f"wk{dc}", name=f"wk{dc}") for dc in range(8)]
            wv_sb = [wp.tile([128, GD], F32R, tag=f"wv{dc}", name=f"wv{dc}") for dc in range(8)]
            wo_sb = [wp.tile([128, D], F32R, tag=f"wo{p}", name=f"wo{p}") for p in range(NPAIR)]
            for dc in range(8):
                nc.sync.dma_start(wq_sb[dc][:], wqT[128 * dc:128 * (dc + 1), :])
                nc.sync.dma_start(wk_sb[dc][:], wkT[128 * dc:128 * (dc + 1), :])
                nc.sync.dma_start(wv_sb[dc][:], wvT[128 * dc:128 * (dc + 1), :])
            for p in range(NPAIR):
                nc.sync.dma_start(wo_sb[p][:], woT[128 * p:128 * (p + 1), :])

            # ---- persistent state --------------------------------------
            ident_f = pp.tile([128, 128], F32, tag="ident_f")
            make_identity(nc, ident_f[:])
            ident_bf = pp.tile([128, 128], BF16, tag="ident_bf")
            nc.vector.tensor_copy(ident_bf[:], ident_f[:])
            # bvec row 64 = 1 (K=1 broadcast weights)
            bv_f32 = pp.tile([66, 64], F32, tag="bv_f32")
            nc.vector.memset(bv_f32[64:65, :], 1.0)
            bvec = pp.tile([66, 64], F32R, tag="bvec")
            nc.vector.tensor_copy(bvec[64:65, :], bv_f32[64:65, :])
            # C/kcum state: f32 master + bf16 matmul operand.
            # caug_st[h]: [64, 66] f32; cols 0-63 C, col 64 kcum, col 65 pad.
            caug_st = [pp.tile([64, 66], F32, tag=f"caug_st{h}", name=f"caug_st{h}")
                       for h in range(HPC)]
            for h in range(HPC):
                nc.vector.memset(caug_st[h][:], 0.0)
                nc.vector.memset(caug_st[h][:, 64:65], 1e-30)
            # caug_bf[h]: [128, 66] bf16; even h: data rows 0-63, rows 64-127
            # stay 0; odd h: data rows 64-127 (written via DMA), rows 0-63
            # stay 0.  K=128 inter matmul works for either half.
            caug_bf = [pp.tile([128, 66], BF16, tag=f"caug_bf{h}", name=f"caug_bf{h}")
                       for h in range(HPC)]
            for h in range(HPC):
                nc.vector.memset(caug_bf[h][:], 0.0)
                if h % 2 == 0:
                    nc.vector.tensor_copy(caug_bf[h][0:64, :], caug_st[h][:])
            caug_bfs = [pp.tile([64, 66], BF16, tag=f"caug_bfs{h}", name=f"caug_bfs{h}")
                        for h in range(1, HPC, 2)]
            for i, h in enumerate(range(1, HPC, 2)):
                nc.vector.tensor_copy(caug_bfs[i][:], caug_st[h][:])
                nc.sync.dma_start(caug_bf[h][64:128, :], caug_bfs[i][:])
            # vaug[s][t4]: [128, 528] bf16, 2 block-parity sets; per head h
            # cols 66h..66h+63 = V, col 66h+64 = 1 (den trick), 66h+65 = 0.
            vaug = [[pp.tile([128, HPC * 66], BF16, tag=f"vaug{s}_{t}", name=f"vaug{s}_{t}")
                     for t in range(4)] for s in range(2)]
            for s in range(2):
                for t4 in range(4):
                    vv = vaug[s][t4][:].rearrange("p (h c) -> p h c", c=66)
                    nc.vector.memset(vv[:, :, 64:65], 1.0)
                    nc.vector.memset(vv[:, :, 65:66], 0.0)
            # ssb[q][t4]: masked S^T chunks, 4 head-parity sets (h%4), bf16.
            # Zero strip rows 64-127 cols c0:c1 preset; data regions are
            # overwritten by every head before its intra matmuls read them.
            ssb = [[pp.tile([128, BT], BF16, tag=f"ssb{q}_{t}", name=f"ssb{q}_{t}")
                    for t in range(4)] for q in range(4)]
            for q in range(4):
                for t4 in range(4):
                    nc.vector.memset(ssb[q][t4][:], 0.0)

            # ---- main loop over coarse blocks ---------------------------
            for ct in range(NBLK):
                t0 = ct * BT
                par2 = ct % 2

                # x^T tiles [d-chunk 128, t 512], straight to f32r
                xsb = [sbp.tile([128, BT], F32R, tag=f"xsb{dc}", name=f"xsb{dc}", bufs=2)
                       for dc in range(8)]
                for dc in range(8):
                    nc.sync.dma_start(
                        xsb[dc][:], xT[128 * dc:128 * (dc + 1), t0:t0 + BT])

                # Q^T projection pair-packed (M=128) + exp -> bf16
                qtu2 = [sbp.tile([128, BT], BF16, tag=f"qtu{p}", name=f"qtu{p}", bufs=2)
                        for p in range(NPAIR)]
                for p in range(NPAIR):
                    pq = ps.tile([128, BT], F32, tag="proj", bufs=2)
                    for dc in range(8):
                        nc.tensor.matmul(
                            pq[:], wq_sb[dc][:, 128 * p:128 * (p + 1)], xsb[dc][:],
                            start=(dc == 0), stop=(dc == 7))
                    nc.scalar.activation(qtu2[p][:], pq[:], EXP)

                # K natural projection per t-chunk + exp -> bf16
                ksb = [sbp.tile([128, GD], BF16, tag=f"ksb{t}", name=f"ksb{t}", bufs=2)
                       for t in range(4)]
                for t4 in range(4):
                    pk = ps.tile([128, GD], F32, tag="proj", bufs=2)
                    for dc in range(8):
                        nc.tensor.matmul(
                            pk[:], xsb[dc][:, 128 * t4:128 * (t4 + 1)], wk_sb[dc][:],
                            start=(dc == 0), stop=(dc == 7))
                    nc.scalar.activation(ksb[t4][:], pk[:], EXP)

                # K^T per pair via bf16 PE transpose: kt2[p] [128, BT]
                kt2 = [sbp.tile([128, BT], BF16, tag=f"kt{p}", name=f"kt{p}", bufs=2)
                       for p in range(NPAIR)]
                for p in range(NPAIR):
                    for t4 in range(4):
                        pt = ps.tile([128, 128], BF16, tag="s", bufs=2)
                        nc.tensor.transpose(
                            pt[:], ksb[t4][:, 128 * p:128 * (p + 1)], ident_bf[:])
                        nc.vector.tensor_copy(
                            kt2[p][:, 128 * t4:128 * (t4 + 1)], pt[:])

                # V projection per t-chunk -> vaug strided cols (bf16)
                for t4 in range(4):
                    pv = ps.tile([128, GD], F32, tag="proj", bufs=2)
                    for dc in range(8):
                        nc.tensor.matmul(
                            pv[:], xsb[dc][:, 128 * t4:128 * (t4 + 1)], wv_sb[dc][:],
                            start=(dc == 0), stop=(dc == 7))
                    vv = vaug[par2][t4][:].rearrange("p (h c) -> p h c", c=66)
                    pvv = pv[:].rearrange("p (h c) -> p h c", c=64)
                    nc.vector.tensor_copy(vv[:, :, 0:64], pvv[:, :, :])

                # ---- attention per head --------------------------------
                xot2 = [sbp.tile([128, BT], F32R, tag=f"xot{p}", name=f"xot{p}", bufs=2)
                        for p in range(NPAIR)]
                for h in range(HPC):
                    p, r, q4 = h // 2, h % 2, h % 4
                    rb = 64 * r
                    # S^T chunks, restricted to needed query range
                    for t4 in range(4):
                        c0 = (2 * t4 + 1) * 64
                        c1 = (2 * t4 + 2) * 64
                        pst = ps.tile([128, BT], F32, tag="s", bufs=2)
                        nc.tensor.matmul(
                            pst[:, c0:BT],
                            kt2[p][rb:rb + 64, 128 * t4:128 * (t4 + 1)],
                            qtu2[p][rb:rb + 64, c0:BT], start=True, stop=True)
                        if (h + t4) % 2 == 0:
                            nc.scalar.copy(ssb[q4][t4][0:64, c0:BT], pst[0:64, c0:BT])
                            if c1 < BT:
                                nc.vector.tensor_copy(
                                    ssb[q4][t4][64:128, c1:BT], pst[64:128, c1:BT])
                        else:
                            nc.vector.tensor_copy(ssb[q4][t4][0:64, c0:BT], pst[0:64, c0:BT])
                            if c1 < BT:
                                nc.scalar.copy(
                                    ssb[q4][t4][64:128, c1:BT], pst[64:128, c1:BT])

                    # OUT group: inter (K=128, bf16) + 4 intra partial-N
                    po = ps.tile([66, BT], F32, tag="o", bufs=2)
                    nc.tensor.matmul(po[:], caug_bf[h][:, :], qtu2[p][:, :],
                                     start=True, stop=False)
                    for t4 in range(4):
                        n0 = (2 * t4 + 1) * 64
                        nc.tensor.matmul(
                            po[0:66, n0:BT],
                            vaug[par2][t4][:, 66 * h:66 * h + 66],
                            ssb[q4][t4][:, n0:BT],
                            start=False, stop=(t4 == 3))

                    # normalize: dinv bcast via K=1 matmul (kcum>=1e-30 so
                    # den>0, no clamp needed)
                    dv = sbp.tile([66, BT], F32R, tag="dv", bufs=4)
                    nc.vector.reciprocal(dv[64:65, :], po[64:65, :])
                    pb = ps.tile([64, BT], F32, tag="b", bufs=1)
                    nc.tensor.matmul(pb[:], bvec[64:65, 0:64], dv[64:65, :],
                                     start=True, stop=True)
                    sbb = sbp.tile([64, BT], F32, tag="sbb", bufs=4)
                    nc.scalar.copy(sbb[:], pb[:])
                    if r == 0:
                        nc.vector.tensor_mul(xot2[p][0:64, :], po[0:64, :], sbb[:])
                    else:
                        xot_o = sbp.tile([64, BT], F32R, tag="xot_o", bufs=2)
                        nc.vector.tensor_mul(xot_o[:], po[0:64, :], sbb[:])
                        nc.gpsimd.dma_start(xot2[p][64:128, :], xot_o[:])

                    # C/kcum update (reads land after this block's inter)
                    pc = ps.tile([64, 66], F32, tag="c", bufs=1)
                    for t4 in range(4):
                        nc.tensor.matmul(
                            pc[:], ksb[t4][:, 64 * h:64 * (h + 1)],
                            vaug[par2][t4][:, 66 * h:66 * h + 66],
                            start=(t4 == 0), stop=(t4 == 3))
                    nc.vector.tensor_add(caug_st[h][:], caug_st[h][:], pc[:])
                    if r == 0:
                        nc.vector.tensor_copy(caug_bf[h][0:64, :], caug_st[h][:])
                    else:
                        cbs = pp.tile([64, 66], BF16, tag=f"caug_bfs{h}",
                                      name=f"cbs{h}", uniquify=True)
                        nc.vector.tensor_copy(cbs[:], caug_st[h][:])
                        nc.gpsimd.dma_start(caug_bf[h][64:128, :], cbs[:])

                # Y projection pair-packed (K=128) + store
                for t4 in range(4):
                    for fc in range(2):
                        py = ps.tile([128, GD], F32, tag="proj", bufs=2)
                        for p in range(NPAIR):
                            nc.tensor.matmul(
                                py[:],
                                xot2[p][:, 128 * t4:128 * (t4 + 1)],
                                wo_sb[p][:, GD * fc:GD * (fc + 1)],
                                start=(p == 0), stop=(p == NPAIR - 1))
                        ysb = sbp.tile([128, GD], F32, tag="ysb", bufs=3)
                        nc.scalar.copy(ysb[:], py[:])
                        nc.sync.dma_start(
                            y[t0 + 128 * t4:t0 + 128 * (t4 + 1),
                              GD * fc:GD * (fc + 1)], ysb[:])

    nc.compile()
    return nc


def _get_nc():
    if "nc" not in _CACHE:
        _CACHE["nc"] = _build()
    return _CACHE["nc"]


def kernel(x, W_qkv, W_out):
    x = np.asarray(x, dtype=np.float32)
    W_qkv = np.asarray(W_qkv, dtype=np.float32)
    W_out = np.asarray(W_out, dtype=np.float32)
    nc = _get_nc()

    xTs = [np.ascontiguousarray(x[b].T) for b in range(B)]
    in_maps = []
    for c in range(NC_CORES):
        b, hg = c // 2, c % 2
        s = slice(hg * GD, (hg + 1) * GD)
        in_maps.append({
            "xT": xTs[b],
            "wqT": np.ascontiguousarray(W_qkv[0 * D:1 * D][s].T),
            "wkT": np.ascontiguousarray(W_qkv[1 * D:2 * D][s].T),
            "wvT": np.ascontiguousarray(W_qkv[2 * D:3 * D][s].T),
            "woT": np.ascontiguousarray(W_out[:, s].T),
        })
    res = run_bass_kernel_spmd(nc, in_maps, core_ids=list(range(NC_CORES)))
    out = np.empty((B, T, D), dtype=np.float32)
    for b in range(B):
        out[b] = res.results[2 * b]["y"] + res.results[2 * b + 1]["y"]
    return out


# revision 5
# speedup vs baseline: 1.9875x; 1.0533x over previous
"""LucidLinearAttention Trainium2 kernel (8-core SPMD), v3.

Sharding: batch b = core//2 (4 batches), head-group hg = core%2 (8 heads each).
Each core: qkv projection for its heads, chunked linear attention over
BT=512 blocks with exact BUCKET=64 causal masking inside the block, partial
output projection. Host sums the two head-group partials per batch.

v3 over v2:
- y stores + weight loads on the Activation HWDGE queue so the SP queue only
  carries x loads: next block's x prefetch is no longer stuck behind the
  current block's y stores (this was a ~4.75us bubble every block).
- Heads processed odds-first so the odd heads' SBUF->SBUF repartition DMAs
  (xot pair packing) complete while the even heads compute.
- S^T emitted 4 heads ahead of the OUT groups; per-head normalize tails
  (bcast/sbb/mul) deferred one head so PE never waits on the recip chain.
- Flexible PSUM->SBUF drains (masked S copies, sbb, ysb) greedily balanced
  across DVE and ACT by estimated cost.

v2 over baseline:
- f32r DRAM tensors, DMA straight into f32r SBUF (no staging copies).
- bf16 attention inner loop (S^T, intra/inter, transposes, C updates).
- Q projection pair-packed (M=128) and Y projection pair-packed (K=128);
  odd heads cross into the packed tiles' rows 64:127 via SBUF->SBUF DMA.
- S^T matmuls restricted to the needed query range per key chunk.
- C/kcum state in f32 (caug_st), re-rounded to bf16 operand each block.
- kcum initialized to 1e-30: den > 0 always, no clamp op needed.
"""
import sys
import numpy as np

for p in ("/opt/trn_rl_repo", "/root/.axon_site/_ro/trn_rl_repo"):
    if p not in sys.path:
        sys.path.insert(0, p)

import concourse.mybir as mybir
import concourse.tile as tile
from concourse import bacc
from concourse.bass_utils import run_bass_kernel_spmd
from concourse.masks import make_identity

F32 = mybir.dt.float32
F32R = mybir.dt.float32r
BF16 = mybir.dt.bfloat16
EXP = mybir.ActivationFunctionType.Exp

B, T, D = 4, 4096, 1024
NH, HD, BUCKET = 16, 64, 64
HPC = 8            # heads per core
GD = HPC * HD      # 512 group dim
NBLK = 8           # coarse blocks
BT = T // NBLK     # 512 rows per block
NPAIR = 4
NC_CORES = 8

_CACHE = {}


def _build():
    nc = bacc.Bacc("TRN2", target_bir_lowering=False, debug=False,
                   num_devices=NC_CORES)
    xT = nc.dram_tensor("xT", [D, T], F32R, kind="ExternalInput").ap()
    wqT = nc.dram_tensor("wqT", [D, GD], F32R, kind="ExternalInput").ap()
    wkT = nc.dram_tensor("wkT", [D, GD], F32R, kind="ExternalInput").ap()
    wvT = nc.dram_tensor("wvT", [D, GD], F32R, kind="ExternalInput").ap()
    woT = nc.dram_tensor("woT", [GD, D], F32R, kind="ExternalInput").ap()
    y = nc.dram_tensor("y", [T, D], F32, kind="ExternalOutput").ap()

    # greedy DVE/ACT balance for flexible PSUM->SBUF drains
    eng_acc = {"dve": 0.0, "act": 0.0}

    def flex_copy(dst, src, nfree):
        cd = 125 + 1.042 * nfree
        ca = 143 + 0.833 * nfree
        if eng_acc["dve"] + cd <= eng_acc["act"] + ca:
            eng_acc["dve"] += cd
            nc.vector.tensor_copy(dst, src)
        else:
            eng_acc["act"] += ca
            nc.scalar.copy(dst, src)

    with tile.TileContext(nc) as tc:
        with nc.allow_low_precision(reason="f32r/bf16 matmul rounding by design"), \
             tc.tile_pool(name="w", bufs=1) as wp, \
             tc.tile_pool(name="per", bufs=1) as pp, \
             tc.tile_pool(name="sb", bufs=1) as sbp, \
             tc.tile_pool(name="ps", bufs=1, space="PSUM") as ps:

            # ---- resident weights on the ACT HWDGE queue ----------------
            wq_sb = [wp.tile([128, GD], F32R, tag=f"wq{dc}", name=f"wq{dc}") for dc in range(8)]
            wk_sb = [wp.tile([128, GD], F32R, tag=f"wk{dc}", name=f"wk{dc}") for dc in range(8)]
            wv_sb = [wp.tile([128, GD], F32R, tag=f"wv{dc}", name=f"wv{dc}") for dc in range(8)]
            wo_sb = [wp.tile([128, D], F32R, tag=f"wo{p}", name=f"wo{p}") for p in range(NPAIR)]
            for dc in range(8):
                nc.scalar.dma_start(wq_sb[dc][:], wqT[128 * dc:128 * (dc + 1), :])
                nc.scalar.dma_start(wk_sb[dc][:], wkT[128 * dc:128 * (dc + 1), :])
                nc.scalar.dma_start(wv_sb[dc][:], wvT[128 * dc:128 * (dc + 1), :])
            for p in range(NPAIR):
                nc.scalar.dma_start(wo_sb[p][:], woT[128 * p:128 * (p + 1), :])

            # ---- persistent state --------------------------------------
            ident_f = pp.tile([128, 128], F32, tag="ident_f")
            make_identity(nc, ident_f[:])
            ident_bf = pp.tile([128, 128], BF16, tag="ident_bf")
            nc.vector.tensor_copy(ident_bf[:], ident_f[:])
            bv_f32 = pp.tile([66, 64], F32, tag="bv_f32")
            nc.vector.memset(bv_f32[64:65, :], 1.0)
            bvec = pp.tile([66, 64], F32R, tag="bvec")
            nc.vector.tensor_copy(bvec[64:65, :], bv_f32[64:65, :])
            # C/kcum state: f32 master + bf16 matmul operand
            caug_st = [pp.tile([64, 66], F32, tag=f"caug_st{h}", name=f"caug_st{h}")
                       for h in range(HPC)]
            for h in range(HPC):
                nc.vector.memset(caug_st[h][:], 0.0)
                nc.vector.memset(caug_st[h][:, 64:65], 1e-30)
            caug_bf = [pp.tile([128, 66], BF16, tag=f"caug_bf{h}", name=f"caug_bf{h}")
                       for h in range(HPC)]
            for h in range(HPC):
                nc.vector.memset(caug_bf[h][:], 0.0)
                if h % 2 == 0:
                    nc.vector.tensor_copy(caug_bf[h][0:64, :], caug_st[h][:])
            for h in range(1, HPC, 2):
                cbs0 = pp.tile([64, 66], BF16, tag=f"caug_bfs{h}", name=f"cbs0_{h}")
                nc.vector.tensor_copy(cbs0[:], caug_st[h][:])
                nc.gpsimd.dma_start(caug_bf[h][64:128, :], cbs0[:])
            # vaug[s][t4]: [128, 528] bf16, 2 block-parity sets
            vaug = [[pp.tile([128, HPC * 66], BF16, tag=f"vaug{s}_{t}", name=f"vaug{s}_{t}")
                     for t in range(4)] for s in range(2)]
            for s in range(2):
                for t4 in range(4):
                    vv = vaug[s][t4][:].rearrange("p (h c) -> p h c", c=66)
                    nc.vector.memset(vv[:, :, 64:65], 1.0)
                    nc.vector.memset(vv[:, :, 65:66], 0.0)
            # ssb[q][t4]: masked S^T chunks, 4 head-parity sets (h%4), bf16
            ssb = [[pp.tile([128, BT], BF16, tag=f"ssb{q}_{t}", name=f"ssb{q}_{t}")
                    for t in range(4)] for q in range(4)]
            for q in range(4):
                for t4 in range(4):
                    nc.vector.memset(ssb[q][t4][:], 0.0)

            # ---- per-block emission helpers ----------------------------
            HEAD_ORDER = [1, 3, 5, 7, 0, 2, 4, 6]

            def emit_st(h, q4, kt2, qtu2):
                """S^T chunks for head h, restricted query range + masked copies."""
                p, r = h // 2, h % 2
                rb = 64 * r
                for t4 in range(4):
                    c0 = (2 * t4 + 1) * 64
                    c1 = (2 * t4 + 2) * 64
                    pst = ps.tile([128, BT], F32, tag="s", name="pst", bufs=2)
                    nc.tensor.matmul(
                        pst[:, c0:BT],
                        kt2[p][rb:rb + 64, 128 * t4:128 * (t4 + 1)],
                        qtu2[p][rb:rb + 64, c0:BT], start=True, stop=True)
                    flex_copy(ssb[q4][t4][0:64, c0:BT], pst[0:64, c0:BT], BT - c0)
                    if c1 < BT:
                        flex_copy(ssb[q4][t4][64:128, c1:BT], pst[64:128, c1:BT],
                                  BT - c1)

            def emit_out(h, q4, qtu2, par2):
                """inter + intra context matmuls -> po [66, BT]."""
                p = h // 2
                po = ps.tile([66, BT], F32, tag="o", name="po", bufs=2)
                nc.tensor.matmul(po[:], caug_bf[h][:, :], qtu2[p][:, :],
                                 start=True, stop=False)
                for t4 in range(4):
                    n0 = (2 * t4 + 1) * 64
                    nc.tensor.matmul(
                        po[0:66, n0:BT],
                        vaug[par2][t4][:, 66 * h:66 * h + 66],
                        ssb[q4][t4][:, n0:BT],
                        start=False, stop=(t4 == 3))
                return po

            def emit_recip(h, po):
                dv = sbp.tile([66, BT], F32R, tag="dv", name="dv", bufs=4)
                nc.vector.reciprocal(dv[64:65, :], po[64:65, :])
                return dv

            def emit_tail(h, po, dv, xot2):
                """bcast + sbb + mul (+ repartition DMA for odd heads)."""
                p, r = h // 2, h % 2
                pb = ps.tile([64, BT], F32, tag="b", name="pb", bufs=1)
                nc.tensor.matmul(pb[:], bvec[64:65, 0:64], dv[64:65, :],
                                 start=True, stop=True)
                sbb = sbp.tile([64, BT], F32, tag="sbb", name="sbb", bufs=4)
                flex_copy(sbb[:], pb[:], BT)
                if r == 0:
                    nc.vector.tensor_mul(xot2[p][0:64, :], po[0:64, :], sbb[:])
                else:
                    xot_o = sbp.tile([64, BT], F32R, tag="xot_o", name="xot_o", bufs=2)
                    nc.vector.tensor_mul(xot_o[:], po[0:64, :], sbb[:])
                    nc.gpsimd.dma_start(xot2[p][64:128, :], xot_o[:])

            def emit_cupd(h, ksb, par2):
                pc = ps.tile([64, 66], F32, tag="c", name="pc", bufs=1)
                for t4 in range(4):
                    nc.tensor.matmul(
                        pc[:], ksb[t4][:, 64 * h:64 * (h + 1)],
                        vaug[par2][t4][:, 66 * h:66 * h + 66],
                        start=(t4 == 0), stop=(t4 == 3))
                nc.vector.tensor_add(caug_st[h][:], caug_st[h][:], pc[:])
                if h % 2 == 0:
                    nc.vector.tensor_copy(caug_bf[h][0:64, :], caug_st[h][:])
                else:
                    cbs = pp.tile([64, 66], BF16, tag=f"caug_bfs{h}", name=f"cbs{h}")
                    nc.vector.tensor_copy(cbs[:], caug_st[h][:])
                    nc.gpsimd.dma_start(caug_bf[h][64:128, :], cbs[:])

            # ---- main loop over coarse blocks ---------------------------
            for ct in range(NBLK):
                t0 = ct * BT
                par2 = ct % 2

                # x^T tiles: SP queue only carries these loads
                xsb = [sbp.tile([128, BT], F32R, tag=f"xsb{dc}", name=f"xsb{dc}", bufs=2)
                       for dc in range(8)]
                for dc in range(8):
                    nc.sync.dma_start(
                        xsb[dc][:], xT[128 * dc:128 * (dc + 1), t0:t0 + BT])

                # Q^T projection pair-packed + exp -> bf16
                qtu2 = [sbp.tile([128, BT], BF16, tag=f"qtu{p}", name=f"qtu{p}", bufs=2)
                        for p in range(NPAIR)]
                for p in range(NPAIR):
                    pq = ps.tile([128, BT], F32, tag="proj", name="pq", bufs=2)
                    for dc in range(8):
                        nc.tensor.matmul(
                            pq[:], wq_sb[dc][:, 128 * p:128 * (p + 1)], xsb[dc][:],
                            start=(dc == 0), stop=(dc == 7))
                    nc.scalar.activation(qtu2[p][:], pq[:], EXP)

                # K natural projection + exp -> bf16
                ksb = [sbp.tile([128, GD], BF16, tag=f"ksb{t}", name=f"ksb{t}", bufs=2)
                       for t in range(4)]
                for t4 in range(4):
                    pk = ps.tile([128, GD], F32, tag="proj", name="pk", bufs=2)
                    for dc in range(8):
                        nc.tensor.matmul(
                            pk[:], xsb[dc][:, 128 * t4:128 * (t4 + 1)], wk_sb[dc][:],
                            start=(dc == 0), stop=(dc == 7))
                    nc.scalar.activation(ksb[t4][:], pk[:], EXP)

                # K^T per pair via bf16 PE transpose
                kt2 = [sbp.tile([128, BT], BF16, tag=f"kt{p}", name=f"kt{p}", bufs=2)
                       for p in range(NPAIR)]
                for p in range(NPAIR):
                    for t4 in range(4):
                        pt = ps.tile([128, 128], BF16, tag="s", name="pt", bufs=2)
                        nc.tensor.transpose(
                            pt[:], ksb[t4][:, 128 * p:128 * (p + 1)], ident_bf[:])
                        nc.vector.tensor_copy(
                            kt2[p][:, 128 * t4:128 * (t4 + 1)], pt[:])

                # V projection -> vaug strided cols (bf16)
                for t4 in range(4):
                    pv = ps.tile([128, GD], F32, tag="proj", name="pv", bufs=2)
                    for dc in range(8):
                        nc.tensor.matmul(
                            pv[:], xsb[dc][:, 128 * t4:128 * (t4 + 1)], wv_sb[dc][:],
                            start=(dc == 0), stop=(dc == 7))
                    vv = vaug[par2][t4][:].rearrange("p (h c) -> p h c", c=66)
                    pvv = pv[:].rearrange("p (h c) -> p h c", c=64)
                    nc.vector.tensor_copy(vv[:, :, 0:64], pvv[:, :, :])

                # ---- attention: S^T 4 heads ahead, tails 1 head behind --
                xot2 = [sbp.tile([128, BT], F32R, tag=f"xot{p}", name=f"xot{p}", bufs=2)
                        for p in range(NPAIR)]
                for i, h in enumerate(HEAD_ORDER[:4]):
                    emit_st(h, i % 4, kt2, qtu2)
                pend = []
                for i, h in enumerate(HEAD_ORDER):
                    po = emit_out(h, i % 4, qtu2, par2)
                    if i + 4 < 8:
                        emit_st(HEAD_ORDER[i + 4], i % 4, kt2, qtu2)
                    dv = emit_recip(h, po)
                    pend.append((h, po, dv))
                    if len(pend) > 1:
                        emit_tail(*pend.pop(0), xot2)
                    emit_cupd(h, ksb, par2)
                emit_tail(*pend.pop(0), xot2)

                # Y projection pair-packed + store on ACT HWDGE queue
                for t4 in range(4):
                    for fc in range(2):
                        py = ps.tile([128, GD], F32, tag="proj", name="py", bufs=2)
                        for p in range(NPAIR):
                            nc.tensor.matmul(
                                py[:],
                                xot2[p][:, 128 * t4:128 * (t4 + 1)],
                                wo_sb[p][:, GD * fc:GD * (fc + 1)],
                                start=(p == 0), stop=(p == NPAIR - 1))
                        ysb = sbp.tile([128, GD], F32, tag="ysb", name="ysb", bufs=3)
                        flex_copy(ysb[:], py[:], GD)
                        nc.scalar.dma_start(
                            y[t0 + 128 * t4:t0 + 128 * (t4 + 1),
                              GD * fc:GD * (fc + 1)], ysb[:])

    nc.compile()
    return nc


def _get_nc():
    if "nc" not in _CACHE:
        _CACHE["nc"] = _build()
    return _CACHE["nc"]


def kernel(x, W_qkv, W_out):
    x = np.asarray(x, dtype=np.float32)
    W_qkv = np.asarray(W_qkv, dtype=np.float32)
    W_out = np.asarray(W_out, dtype=np.float32)
    nc = _get_nc()

    xTs = [np.ascontiguousarray(x[b].T) for b in range(B)]
    in_maps = []
    for c in range(NC_CORES):
        b, hg = c // 2, c % 2
        s = slice(hg * GD, (hg + 1) * GD)
        in_maps.append({
            "xT": xTs[b],
            "wqT": np.ascontiguousarray(W_qkv[0 * D:1 * D][s].T),
            "wkT": np.ascontiguousarray(W_qkv[1 * D:2 * D][s].T),
            "wvT": np.ascontiguousarray(W_qkv[2 * D:3 * D][s].T),
            "woT": np.ascontiguousarray(W_out[:, s].T),
        })
    res = run_bass_kernel_spmd(nc, in_maps, core_ids=list(range(NC_CORES)))
    out = np.empty((B, T, D), dtype=np.float32)
    for b in range(B):
        out[b] = res.results[2 * b]["y"] + res.results[2 * b + 1]["y"]
    return out


# revision 6
# speedup vs baseline: 2.0472x; 1.0300x over previous
"""LucidLinearAttention Trainium2 kernel (8-core SPMD), v3.

Sharding: batch b = core//2 (4 batches), head-group hg = core%2 (8 heads each).
Each core: qkv projection for its heads, chunked linear attention over
BT=512 blocks with exact BUCKET=64 causal masking inside the block, partial
output projection. Host sums the two head-group partials per batch.

v3 over v2:
- y stores + weight loads on the Activation HWDGE queue so the SP queue only
  carries x loads: next block's x prefetch is no longer stuck behind the
  current block's y stores (this was a ~4.75us bubble every block).
- Heads processed odds-first so the odd heads' SBUF->SBUF repartition DMAs
  (xot pair packing) complete while the even heads compute.
- S^T emitted 4 heads ahead of the OUT groups; per-head normalize tails
  (bcast/sbb/mul) deferred one head so PE never waits on the recip chain.
- Flexible PSUM->SBUF drains (masked S copies, sbb, ysb) greedily balanced
  across DVE and ACT by estimated cost.

v2 over baseline:
- f32r DRAM tensors, DMA straight into f32r SBUF (no staging copies).
- bf16 attention inner loop (S^T, intra/inter, transposes, C updates).
- Q projection pair-packed (M=128) and Y projection pair-packed (K=128);
  odd heads cross into the packed tiles' rows 64:127 via SBUF->SBUF DMA.
- S^T matmuls restricted to the needed query range per key chunk.
- C/kcum state in f32 (caug_st), re-rounded to bf16 operand each block.
- kcum initialized to 1e-30: den > 0 always, no clamp op needed.
"""
import sys
import numpy as np

for p in ("/opt/trn_rl_repo", "/root/.axon_site/_ro/trn_rl_repo"):
    if p not in sys.path:
        sys.path.insert(0, p)

import concourse.mybir as mybir
import concourse.tile as tile
from concourse import bacc
from concourse.bass_utils import run_bass_kernel_spmd
from concourse.masks import make_identity

F32 = mybir.dt.float32
F32R = mybir.dt.float32r
BF16 = mybir.dt.bfloat16
EXP = mybir.ActivationFunctionType.Exp

B, T, D = 4, 4096, 1024
NH, HD, BUCKET = 16, 64, 64
HPC = 8            # heads per core
GD = HPC * HD      # 512 group dim
NBLK = 8           # coarse blocks
BT = T // NBLK     # 512 rows per block
NPAIR = 4
NC_CORES = 8

_CACHE = {}


def _build():
    nc = bacc.Bacc("TRN2", target_bir_lowering=False, debug=False,
                   num_devices=NC_CORES)
    xT = nc.dram_tensor("xT", [D, T], F32R, kind="ExternalInput").ap()
    wqT = nc.dram_tensor("wqT", [D, GD], F32R, kind="ExternalInput").ap()
    wkT = nc.dram_tensor("wkT", [D, GD], F32R, kind="ExternalInput").ap()
    wvT = nc.dram_tensor("wvT", [D, GD], F32R, kind="ExternalInput").ap()
    woT = nc.dram_tensor("woT", [GD, D], F32R, kind="ExternalInput").ap()
    y = nc.dram_tensor("y", [T, D], F32, kind="ExternalOutput").ap()

    # greedy DVE/ACT balance for flexible PSUM->SBUF drains
    eng_acc = {"dve": 0.0, "act": 0.0}

    def flex_copy(dst, src, nfree):
        cd = 125 + 1.042 * nfree
        ca = 143 + 0.833 * nfree
        if eng_acc["dve"] + cd <= eng_acc["act"] + ca:
            eng_acc["dve"] += cd
            nc.vector.tensor_copy(dst, src)
        else:
            eng_acc["act"] += ca
            nc.scalar.copy(dst, src)

    def acc(engine, cost):
        eng_acc[engine] += cost

    with tile.TileContext(nc) as tc:
        with nc.allow_low_precision(reason="f32r/bf16 matmul rounding by design"), \
             tc.tile_pool(name="w", bufs=1) as wp, \
             tc.tile_pool(name="per", bufs=1) as pp, \
             tc.tile_pool(name="sb", bufs=1) as sbp, \
             tc.tile_pool(name="ps", bufs=1, space="PSUM") as ps:

            # ---- resident weights on the ACT HWDGE queue ----------------
            wq_sb = [wp.tile([128, GD], F32R, tag=f"wq{dc}", name=f"wq{dc}") for dc in range(8)]
            wk_sb = [wp.tile([128, GD], F32R, tag=f"wk{dc}", name=f"wk{dc}") for dc in range(8)]
            wv_sb = [wp.tile([128, GD], F32R, tag=f"wv{dc}", name=f"wv{dc}") for dc in range(8)]
            wo_sb = [wp.tile([128, D], F32R, tag=f"wo{p}", name=f"wo{p}") for p in range(NPAIR)]
            for dc in range(8):
                nc.scalar.dma_start(wq_sb[dc][:], wqT[128 * dc:128 * (dc + 1), :])
                nc.scalar.dma_start(wk_sb[dc][:], wkT[128 * dc:128 * (dc + 1), :])
                nc.scalar.dma_start(wv_sb[dc][:], wvT[128 * dc:128 * (dc + 1), :])
            for p in range(NPAIR):
                nc.scalar.dma_start(wo_sb[p][:], woT[128 * p:128 * (p + 1), :])

            # ---- persistent state --------------------------------------
            ident_f = pp.tile([128, 128], F32, tag="ident_f")
            make_identity(nc, ident_f[:])
            ident_bf = pp.tile([128, 128], BF16, tag="ident_bf")
            nc.vector.tensor_copy(ident_bf[:], ident_f[:])
            bv_f32 = pp.tile([66, 64], F32, tag="bv_f32")
            nc.vector.memset(bv_f32[64:65, :], 1.0)
            bvec = pp.tile([66, 64], F32R, tag="bvec")
            nc.vector.tensor_copy(bvec[64:65, :], bv_f32[64:65, :])
            # C/kcum state: f32 master + bf16 matmul operand
            caug_st = [pp.tile([64, 66], F32, tag=f"caug_st{h}", name=f"caug_st{h}")
                       for h in range(HPC)]
            for h in range(HPC):
                nc.gpsimd.memset(caug_st[h][:], 0.0)
                nc.gpsimd.memset(caug_st[h][:, 64:65], 1e-30)
            caug_bf = [pp.tile([128, 66], BF16, tag=f"caug_bf{h}", name=f"caug_bf{h}")
                       for h in range(HPC)]
            for h in range(HPC):
                nc.gpsimd.memset(caug_bf[h][:], 0.0)
                if h % 2 == 0:
                    nc.vector.tensor_copy(caug_bf[h][0:64, :], caug_st[h][:])
            for h in range(1, HPC, 2):
                cbs0 = pp.tile([64, 66], BF16, tag=f"caug_bfs{h}", name=f"cbs0_{h}")
                nc.vector.tensor_copy(cbs0[:], caug_st[h][:])
                nc.gpsimd.dma_start(caug_bf[h][64:128, :], cbs0[:])
            # vaug[s][t4]: [128, 528] bf16, 2 block-parity sets
            vaug = [[pp.tile([128, HPC * 66], BF16, tag=f"vaug{s}_{t}", name=f"vaug{s}_{t}")
                     for t in range(4)] for s in range(2)]
            for s in range(2):
                for t4 in range(4):
                    vv = vaug[s][t4][:].rearrange("p (h c) -> p h c", c=66)
                    nc.gpsimd.memset(vv[:, :, 64:65], 1.0)
                    nc.gpsimd.memset(vv[:, :, 65:66], 0.0)
            # ssb[q][t4]: masked S^T chunks, 4 head-parity sets (h%4), bf16
            ssb = [[pp.tile([128, BT], BF16, tag=f"ssb{q}_{t}", name=f"ssb{q}_{t}")
                    for t in range(4)] for q in range(4)]
            for q in range(4):
                for t4 in range(4):
                    nc.gpsimd.memset(ssb[q][t4][:], 0.0)

            # ---- per-block emission helpers ----------------------------
            HEAD_ORDER = [1, 3, 5, 7, 0, 2, 4, 6]

            def emit_st(h, q4, kt2, qtu2):
                """S^T chunks for head h, restricted query range + masked copies."""
                p, r = h // 2, h % 2
                rb = 64 * r
                for t4 in range(4):
                    c0 = (2 * t4 + 1) * 64
                    c1 = (2 * t4 + 2) * 64
                    pst = ps.tile([128, BT], F32, tag="s", name="pst", bufs=2)
                    nc.tensor.matmul(
                        pst[:, c0:BT],
                        kt2[p][rb:rb + 64, 128 * t4:128 * (t4 + 1)],
                        qtu2[p][rb:rb + 64, c0:BT], start=True, stop=True)
                    flex_copy(ssb[q4][t4][0:64, c0:BT], pst[0:64, c0:BT], BT - c0)
                    if c1 < BT:
                        flex_copy(ssb[q4][t4][64:128, c1:BT], pst[64:128, c1:BT],
                                  BT - c1)

            def emit_out(h, q4, qtu2, par2):
                """inter + intra context matmuls -> po [66, BT]."""
                p = h // 2
                po = ps.tile([66, BT], F32, tag="o", name="po", bufs=2)
                nc.tensor.matmul(po[:], caug_bf[h][:, :], qtu2[p][:, :],
                                 start=True, stop=False)
                for t4 in range(4):
                    n0 = (2 * t4 + 1) * 64
                    nc.tensor.matmul(
                        po[0:66, n0:BT],
                        vaug[par2][t4][:, 66 * h:66 * h + 66],
                        ssb[q4][t4][:, n0:BT],
                        start=False, stop=(t4 == 3))
                return po

            def emit_recip(h, po):
                """drain po rows 0:64 to SBUF + reciprocal of den row."""
                dv = sbp.tile([66, BT], F32R, tag="dv", name="dv", bufs=4)
                nc.vector.reciprocal(dv[64:65, :], po[64:65, :])
                acc("dve", 658)
                po_sb = sbp.tile([64, BT], F32, tag="po_sb", name="po_sb", bufs=4)
                flex_copy(po_sb[:], po[0:64, :], BT)
                return dv, po_sb

            def emit_tail(h, dv, po_sb, xot2):
                """bcast + mul (+ repartition DMA for odd heads)."""
                p, r = h // 2, h % 2
                pb = ps.tile([64, BT], F32, tag="bc", name="pb", bufs=2)
                nc.tensor.matmul(pb[:], bvec[64:65, 0:64], dv[64:65, :],
                                 start=True, stop=True)
                if r == 0:
                    nc.vector.tensor_mul(xot2[p][0:64, :], pb[:], po_sb[:])
                    acc("dve", 658)
                else:
                    xot_o = sbp.tile([64, BT], F32R, tag="xot_o", name="xot_o", bufs=2)
                    nc.vector.tensor_mul(xot_o[:], pb[:], po_sb[:])
                    acc("dve", 658)
                    nc.gpsimd.dma_start(xot2[p][64:128, :], xot_o[:])

            def emit_cupd(h, ksb, par2):
                pc = ps.tile([64, 66], F32, tag="bc", name="pc", bufs=2)
                for t4 in range(4):
                    nc.tensor.matmul(
                        pc[:], ksb[t4][:, 64 * h:64 * (h + 1)],
                        vaug[par2][t4][:, 66 * h:66 * h + 66],
                        start=(t4 == 0), stop=(t4 == 3))
                nc.vector.tensor_add(caug_st[h][:], caug_st[h][:], pc[:])
                acc("dve", 194)
                if h % 2 == 0:
                    nc.vector.tensor_copy(caug_bf[h][0:64, :], caug_st[h][:])
                    acc("dve", 194)
                else:
                    cbs = pp.tile([64, 66], BF16, tag=f"caug_bfs{h}", name=f"cbs{h}")
                    nc.vector.tensor_copy(cbs[:], caug_st[h][:])
                    acc("dve", 194)
                    nc.gpsimd.dma_start(caug_bf[h][64:128, :], cbs[:])

            # ---- main loop over coarse blocks ---------------------------
            for ct in range(NBLK):
                t0 = ct * BT
                par2 = ct % 2

                # x^T tiles: SP queue only carries these loads
                xsb = [sbp.tile([128, BT], F32R, tag=f"xsb{dc}", name=f"xsb{dc}", bufs=2)
                       for dc in range(8)]
                for dc in range(8):
                    nc.sync.dma_start(
                        xsb[dc][:], xT[128 * dc:128 * (dc + 1), t0:t0 + BT])

                # Q^T projection pair-packed + exp -> bf16
                qtu2 = [sbp.tile([128, BT], BF16, tag=f"qtu{p}", name=f"qtu{p}", bufs=2)
                        for p in range(NPAIR)]
                for p in range(NPAIR):
                    pq = ps.tile([128, BT], F32, tag="proj", name="pq", bufs=2)
                    for dc in range(8):
                        nc.tensor.matmul(
                            pq[:], wq_sb[dc][:, 128 * p:128 * (p + 1)], xsb[dc][:],
                            start=(dc == 0), stop=(dc == 7))
                    nc.scalar.activation(qtu2[p][:], pq[:], EXP)
                    acc("act", 612)

                # K natural projection + exp -> bf16
                ksb = [sbp.tile([128, GD], BF16, tag=f"ksb{t}", name=f"ksb{t}", bufs=2)
                       for t in range(4)]
                for t4 in range(4):
                    pk = ps.tile([128, GD], F32, tag="proj", name="pk", bufs=2)
                    for dc in range(8):
                        nc.tensor.matmul(
                            pk[:], xsb[dc][:, 128 * t4:128 * (t4 + 1)], wk_sb[dc][:],
                            start=(dc == 0), stop=(dc == 7))
                    nc.scalar.activation(ksb[t4][:], pk[:], EXP)
                    acc("act", 612)

                # K^T per pair via bf16 PE transpose
                kt2 = [sbp.tile([128, BT], BF16, tag=f"kt{p}", name=f"kt{p}", bufs=2)
                       for p in range(NPAIR)]
                for p in range(NPAIR):
                    for t4 in range(4):
                        pt = ps.tile([128, 128], BF16, tag="s", name="pt", bufs=2)
                        nc.tensor.transpose(
                            pt[:], ksb[t4][:, 128 * p:128 * (p + 1)], ident_bf[:])
                        flex_copy(kt2[p][:, 128 * t4:128 * (t4 + 1)], pt[:], 128)

                # V projection -> vaug strided cols (bf16)
                for t4 in range(4):
                    pv = ps.tile([128, GD], F32, tag="proj", name="pv", bufs=2)
                    for dc in range(8):
                        nc.tensor.matmul(
                            pv[:], xsb[dc][:, 128 * t4:128 * (t4 + 1)], wv_sb[dc][:],
                            start=(dc == 0), stop=(dc == 7))
                    vv = vaug[par2][t4][:].rearrange("p (h c) -> p h c", c=66)
                    pvv = pv[:].rearrange("p (h c) -> p h c", c=64)
                    flex_copy(vv[:, :, 0:64], pvv[:, :, :], BT)

                # ---- attention: S^T 4 heads ahead, tails 1 head behind --
                xot2 = [sbp.tile([128, BT], F32R, tag=f"xot{p}", name=f"xot{p}", bufs=2)
                        for p in range(NPAIR)]
                for i, h in enumerate(HEAD_ORDER[:4]):
                    emit_st(h, i % 4, kt2, qtu2)
                pend = []
                for i, h in enumerate(HEAD_ORDER):
                    po = emit_out(h, i % 4, qtu2, par2)
                    if i + 4 < 8:
                        emit_st(HEAD_ORDER[i + 4], i % 4, kt2, qtu2)
                    dv, po_sb = emit_recip(h, po)
                    pend.append((h, dv, po_sb))
                    if len(pend) > 1:
                        emit_tail(*pend.pop(0), xot2)
                    emit_cupd(h, ksb, par2)
                emit_tail(*pend.pop(0), xot2)

                # Y projection pair-packed + store on ACT HWDGE queue
                for t4 in range(4):
                    for fc in range(2):
                        py = ps.tile([128, GD], F32, tag="proj", name="py", bufs=2)
                        for p in range(NPAIR):
                            nc.tensor.matmul(
                                py[:],
                                xot2[p][:, 128 * t4:128 * (t4 + 1)],
                                wo_sb[p][:, GD * fc:GD * (fc + 1)],
                                start=(p == 0), stop=(p == NPAIR - 1))
                        ysb = sbp.tile([128, GD], F32, tag="ysb", name="ysb", bufs=3)
                        flex_copy(ysb[:], py[:], GD)
                        nc.scalar.dma_start(
                            y[t0 + 128 * t4:t0 + 128 * (t4 + 1),
                              GD * fc:GD * (fc + 1)], ysb[:])

    nc.compile()
    return nc


def _get_nc():
    if "nc" not in _CACHE:
        _CACHE["nc"] = _build()
    return _CACHE["nc"]


def kernel(x, W_qkv, W_out):
    x = np.asarray(x, dtype=np.float32)
    W_qkv = np.asarray(W_qkv, dtype=np.float32)
    W_out = np.asarray(W_out, dtype=np.float32)
    nc = _get_nc()

    xTs = [np.ascontiguousarray(x[b].T) for b in range(B)]
    in_maps = []
    for c in range(NC_CORES):
        b, hg = c // 2, c % 2
        s = slice(hg * GD, (hg + 1) * GD)
        in_maps.append({
            "xT": xTs[b],
            "wqT": np.ascontiguousarray(W_qkv[0 * D:1 * D][s].T),
            "wkT": np.ascontiguousarray(W_qkv[1 * D:2 * D][s].T),
            "wvT": np.ascontiguousarray(W_qkv[2 * D:3 * D][s].T),
            "woT": np.ascontiguousarray(W_out[:, s].T),
        })
    res = run_bass_kernel_spmd(nc, in_maps, core_ids=list(range(NC_CORES)))
    out = np.empty((B, T, D), dtype=np.float32)
    for b in range(B):
        out[b] = res.results[2 * b]["y"] + res.results[2 * b + 1]["y"]
    return out


# revision 8
# speedup vs baseline: 2.5824x; 1.2614x over previous
"""LucidLinearAttention Trainium2 kernel (8-core SPMD), v3.

Sharding: batch b = core//2 (4 batches), head-group hg = core%2 (8 heads each).
Each core: qkv projection for its heads, chunked linear attention over
BT=512 blocks with exact BUCKET=64 causal masking inside the block, partial
output projection. Host sums the two head-group partials per batch.

v3 over v2:
- y stores + weight loads on the Activation HWDGE queue so the SP queue only
  carries x loads: next block's x prefetch is no longer stuck behind the
  current block's y stores (this was a ~4.75us bubble every block).
- Heads processed odds-first so the odd heads' SBUF->SBUF repartition DMAs
  (xot pair packing) complete while the even heads compute.
- S^T emitted 4 heads ahead of the OUT groups; per-head normalize tails
  (bcast/sbb/mul) deferred one head so PE never waits on the recip chain.
- Flexible PSUM->SBUF drains (masked S copies, sbb, ysb) greedily balanced
  across DVE and ACT by estimated cost.

v2 over baseline:
- f32r DRAM tensors, DMA straight into f32r SBUF (no staging copies).
- bf16 attention inner loop (S^T, intra/inter, transposes, C updates).
- Q projection pair-packed (M=128) and Y projection pair-packed (K=128);
  odd heads cross into the packed tiles' rows 64:127 via SBUF->SBUF DMA.
- S^T matmuls restricted to the needed query range per key chunk.
- C/kcum state in f32 (caug_st), re-rounded to bf16 operand each block.
- kcum initialized to 1e-30: den > 0 always, no clamp op needed.
"""
import sys
import numpy as np

for p in ("/opt/trn_rl_repo", "/root/.axon_site/_ro/trn_rl_repo"):
    if p not in sys.path:
        sys.path.insert(0, p)

import concourse.mybir as mybir
import concourse.tile as tile
from concourse import bacc
from concourse.bass_utils import run_bass_kernel_spmd
from concourse.masks import make_identity

F32 = mybir.dt.float32
F32R = mybir.dt.float32r
BF16 = mybir.dt.bfloat16
EXP = mybir.ActivationFunctionType.Exp

B, T, D = 4, 4096, 1024
NH, HD, BUCKET = 16, 64, 64
HPC = 8            # heads per core
GD = HPC * HD      # 512 group dim
NBLK = 8           # coarse blocks
BT = T // NBLK     # 512 rows per block
NPAIR = 4
NC_CORES = 8

_CACHE = {}


def _build():
    nc = bacc.Bacc("TRN2", target_bir_lowering=False, debug=False,
                   num_devices=NC_CORES)
    xT = nc.dram_tensor("xT", [D, T], F32R, kind="ExternalInput").ap()
    wqT = nc.dram_tensor("wqT", [D, GD], F32R, kind="ExternalInput").ap()
    wkT = nc.dram_tensor("wkT", [D, GD], F32R, kind="ExternalInput").ap()
    wvT = nc.dram_tensor("wvT", [D, GD], F32R, kind="ExternalInput").ap()
    woT = nc.dram_tensor("woT", [GD, D], F32R, kind="ExternalInput").ap()
    y = nc.dram_tensor("y", [T, D], F32, kind="ExternalOutput").ap()

    # greedy DVE/ACT balance for flexible PSUM->SBUF drains
    eng_acc = {"dve": 0.0, "act": 0.0}

    def flex_copy(dst, src, nfree):
        cd = 125 + 1.042 * nfree
        ca = 143 + 0.833 * nfree
        if eng_acc["dve"] + cd <= eng_acc["act"] + ca:
            eng_acc["dve"] += cd
            nc.vector.tensor_copy(dst, src)
        else:
            eng_acc["act"] += ca
            nc.scalar.copy(dst, src)

    def acc(engine, cost):
        eng_acc[engine] += cost

    with tile.TileContext(nc) as tc:
        with nc.allow_low_precision(reason="f32r/bf16 matmul rounding by design"), \
             tc.tile_pool(name="w", bufs=1) as wp, \
             tc.tile_pool(name="per", bufs=1) as pp, \
             tc.tile_pool(name="sb", bufs=1) as sbp, \
             tc.tile_pool(name="ps", bufs=1, space="PSUM") as ps:

            # ---- resident weights on the ACT HWDGE queue ----------------
            wq_sb = [wp.tile([128, GD], F32R, tag=f"wq{dc}", name=f"wq{dc}") for dc in range(8)]
            wk_sb = [wp.tile([128, GD], F32R, tag=f"wk{dc}", name=f"wk{dc}") for dc in range(8)]
            wv_sb = [wp.tile([128, GD], F32R, tag=f"wv{dc}", name=f"wv{dc}") for dc in range(8)]
            wo_sb = [wp.tile([128, D], F32R, tag=f"wo{p}", name=f"wo{p}") for p in range(NPAIR)]
            for dc in range(8):
                nc.scalar.dma_start(wq_sb[dc][:], wqT[128 * dc:128 * (dc + 1), :])
                nc.scalar.dma_start(wk_sb[dc][:], wkT[128 * dc:128 * (dc + 1), :])
                nc.scalar.dma_start(wv_sb[dc][:], wvT[128 * dc:128 * (dc + 1), :])
            for p in range(NPAIR):
                nc.scalar.dma_start(wo_sb[p][:], woT[128 * p:128 * (p + 1), :])

            # ---- persistent state --------------------------------------
            ident_f = pp.tile([128, 128], F32, tag="ident_f")
            make_identity(nc, ident_f[:])
            ident_bf = pp.tile([128, 128], BF16, tag="ident_bf")
            nc.vector.tensor_copy(ident_bf[:], ident_f[:])
            bv_f32 = pp.tile([66, 64], F32, tag="bv_f32")
            nc.vector.memset(bv_f32[64:65, :], 1.0)
            bvec = pp.tile([66, 64], F32R, tag="bvec")
            nc.vector.tensor_copy(bvec[64:65, :], bv_f32[64:65, :])
            # C/kcum state: f32 master + bf16 matmul operand
            caug_st = [pp.tile([64, 66], F32, tag=f"caug_st{h}", name=f"caug_st{h}")
                       for h in range(HPC)]
            for h in range(HPC):
                nc.gpsimd.memset(caug_st[h][:], 0.0)
                nc.gpsimd.memset(caug_st[h][:, 64:65], 1e-30)
            caug_bf = [pp.tile([128, 66], BF16, tag=f"caug_bf{h}", name=f"caug_bf{h}")
                       for h in range(HPC)]
            for h in range(HPC):
                nc.gpsimd.memset(caug_bf[h][:], 0.0)
                if h % 2 == 0:
                    nc.vector.tensor_copy(caug_bf[h][0:64, :], caug_st[h][:])
            for h in range(1, HPC, 2):
                cbs0 = pp.tile([64, 66], BF16, tag=f"caug_bfs{h}", name=f"cbs0_{h}")
                nc.vector.tensor_copy(cbs0[:], caug_st[h][:])
                nc.gpsimd.dma_start(caug_bf[h][64:128, :], cbs0[:])
            # vaug[s][t4]: [128, 528] bf16, 2 block-parity sets
            vaug = [[pp.tile([128, HPC * 66], BF16, tag=f"vaug{s}_{t}", name=f"vaug{s}_{t}")
                     for t in range(4)] for s in range(2)]
            for s in range(2):
                for t4 in range(4):
                    vv = vaug[s][t4][:].rearrange("p (h c) -> p h c", c=66)
                    nc.gpsimd.memset(vv[:, :, 64:65], 1.0)
                    nc.gpsimd.memset(vv[:, :, 65:66], 0.0)
            # ssb[q][t4]: masked S^T chunks, 4 head-parity sets (h%4), bf16
            ssb = [[pp.tile([128, BT], BF16, tag=f"ssb{q}_{t}", name=f"ssb{q}_{t}")
                    for t in range(4)] for q in range(4)]
            for q in range(4):
                for t4 in range(4):
                    nc.gpsimd.memset(ssb[q][t4][:], 0.0)

            # ---- per-block emission helpers ----------------------------
            HEAD_ORDER = [1, 3, 5, 7, 0, 2, 4, 6]

            def emit_st(h, q4, kt2, qtu2):
                """S^T chunks for head h, restricted query range + masked copies."""
                p, r = h // 2, h % 2
                rb = 64 * r
                for t4 in range(4):
                    c0 = (2 * t4 + 1) * 64
                    c1 = (2 * t4 + 2) * 64
                    pst = ps.tile([128, BT], F32, tag="s", name="pst", bufs=2)
                    nc.tensor.matmul(
                        pst[:, c0:BT],
                        kt2[p][rb:rb + 64, 128 * t4:128 * (t4 + 1)],
                        qtu2[p][rb:rb + 64, c0:BT], start=True, stop=True)
                    flex_copy(ssb[q4][t4][0:64, c0:BT], pst[0:64, c0:BT], BT - c0)
                    if c1 < BT:
                        flex_copy(ssb[q4][t4][64:128, c1:BT], pst[64:128, c1:BT],
                                  BT - c1)

            def emit_out(h, q4, qtu2, par2):
                """inter + intra context matmuls -> po [66, BT]."""
                p = h // 2
                po = ps.tile([66, BT], F32, tag="o", name="po", bufs=2)
                nc.tensor.matmul(po[:], caug_bf[h][:, :], qtu2[p][:, :],
                                 start=True, stop=False)
                for t4 in range(4):
                    n0 = (2 * t4 + 1) * 64
                    nc.tensor.matmul(
                        po[0:66, n0:BT],
                        vaug[par2][t4][:, 66 * h:66 * h + 66],
                        ssb[q4][t4][:, n0:BT],
                        start=False, stop=(t4 == 3))
                return po

            def emit_recip(h, po):
                """drain po rows 0:64 to SBUF + reciprocal of den row."""
                dv = sbp.tile([66, BT], F32R, tag="dv", name="dv", bufs=4)
                nc.vector.reciprocal(dv[64:65, :], po[64:65, :])
                acc("dve", 658)
                po_sb = sbp.tile([64, BT], F32, tag="po_sb", name="po_sb", bufs=4)
                flex_copy(po_sb[:], po[0:64, :], BT)
                return dv, po_sb

            def emit_tail(h, dv, po_sb, xot2):
                """bcast + mul (+ repartition DMA for odd heads)."""
                p, r = h // 2, h % 2
                pb = ps.tile([64, BT], F32, tag="bc", name="pb", bufs=2)
                nc.tensor.matmul(pb[:], bvec[64:65, 0:64], dv[64:65, :],
                                 start=True, stop=True)
                if r == 0:
                    nc.vector.tensor_mul(xot2[p][0:64, :], pb[:], po_sb[:])
                    acc("dve", 658)
                else:
                    xot_o = sbp.tile([64, BT], F32R, tag="xot_o", name="xot_o", bufs=2)
                    nc.vector.tensor_mul(xot_o[:], pb[:], po_sb[:])
                    acc("dve", 658)
                    nc.gpsimd.dma_start(xot2[p][64:128, :], xot_o[:])

            def emit_cupd(h, ksb, par2):
                pc = ps.tile([64, 66], F32, tag="bc", name="pc", bufs=2)
                for t4 in range(4):
                    nc.tensor.matmul(
                        pc[:], ksb[t4][:, 64 * h:64 * (h + 1)],
                        vaug[par2][t4][:, 66 * h:66 * h + 66],
                        start=(t4 == 0), stop=(t4 == 3))
                nc.vector.tensor_add(caug_st[h][:], caug_st[h][:], pc[:])
                acc("dve", 194)
                if h % 2 == 0:
                    nc.vector.tensor_copy(caug_bf[h][0:64, :], caug_st[h][:])
                    acc("dve", 194)
                else:
                    cbs = pp.tile([64, 66], BF16, tag=f"caug_bfs{h}", name=f"cbs{h}")
                    nc.vector.tensor_copy(cbs[:], caug_st[h][:])
                    acc("dve", 194)
                    nc.gpsimd.dma_start(caug_bf[h][64:128, :], cbs[:])

            # ---- main loop over coarse blocks, software-pipelined ------
            # Iteration ct emits: x loads(ct) + interleaved [attention+Y of
            # block ct-1] and [projections of block ct].  PE then always has
            # independent projection matmuls available while the attention
            # dependency chains (ssb copies, recip/bcast/mul) resolve.
            prev = None
            for ct in range(NBLK + 1):
                proj_units = []
                if ct < NBLK:
                    t0 = ct * BT
                    par2 = ct % 2
                    xsb = [sbp.tile([128, BT], F32R, tag=f"xsb{dc}", name=f"xsb{dc}", bufs=2)
                           for dc in range(8)]
                    for dc in range(8):
                        nc.sync.dma_start(
                            xsb[dc][:], xT[128 * dc:128 * (dc + 1), t0:t0 + BT])
                    qtu2 = [sbp.tile([128, BT], BF16, tag=f"qtu{p}", name=f"qtu{p}", bufs=2)
                            for p in range(NPAIR)]
                    ksb = [sbp.tile([128, GD], BF16, tag=f"ksb{t}", name=f"ksb{t}", bufs=2)
                           for t in range(4)]
                    kt2 = [sbp.tile([128, BT], BF16, tag=f"kt{p}", name=f"kt{p}", bufs=2)
                           for p in range(NPAIR)]

                    def mk_q(p, qtu2=qtu2, xsb=xsb):
                        def u():
                            pq = ps.tile([128, BT], F32, tag="proj", name="pq", bufs=2)
                            for dc in range(8):
                                nc.tensor.matmul(
                                    pq[:], wq_sb[dc][:, 128 * p:128 * (p + 1)], xsb[dc][:],
                                    start=(dc == 0), stop=(dc == 7))
                            nc.scalar.activation(qtu2[p][:], pq[:], EXP)
                            acc("act", 612)
                        return u

                    def mk_k(t4, ksb=ksb, xsb=xsb):
                        def u():
                            pk = ps.tile([128, GD], F32, tag="proj", name="pk", bufs=2)
                            for dc in range(8):
                                nc.tensor.matmul(
                                    pk[:], xsb[dc][:, 128 * t4:128 * (t4 + 1)], wk_sb[dc][:],
                                    start=(dc == 0), stop=(dc == 7))
                            nc.scalar.activation(ksb[t4][:], pk[:], EXP)
                            acc("act", 612)
                        return u

                    def mk_tr(t4, ksb=ksb, kt2=kt2):
                        # transpose chunk t4 for ALL pairs (reads only ksb[t4])
                        def u():
                            for p in range(NPAIR):
                                pt = ps.tile([128, 128], BF16, tag="s", name="pt", bufs=2)
                                nc.tensor.transpose(
                                    pt[:], ksb[t4][:, 128 * p:128 * (p + 1)], ident_bf[:])
                                flex_copy(kt2[p][:, 128 * t4:128 * (t4 + 1)], pt[:], 128)
                        return u

                    def mk_v(t4, xsb=xsb, par2=par2):
                        def u():
                            pv = ps.tile([128, GD], F32, tag="proj", name="pv", bufs=2)
                            for dc in range(8):
                                nc.tensor.matmul(
                                    pv[:], xsb[dc][:, 128 * t4:128 * (t4 + 1)], wv_sb[dc][:],
                                    start=(dc == 0), stop=(dc == 7))
                            vv = vaug[par2][t4][:].rearrange("p (h c) -> p h c", c=66)
                            pvv = pv[:].rearrange("p (h c) -> p h c", c=64)
                            flex_copy(vv[:, :, 0:64], pvv[:, :, :], BT)
                        return u

                    proj_units = [mk_q(0), mk_q(1), mk_q(2), mk_q(3),
                                  mk_k(0), mk_tr(0), mk_k(1), mk_tr(1),
                                  mk_k(2), mk_tr(2), mk_k(3), mk_tr(3),
                                  mk_v(0), mk_v(1), mk_v(2), mk_v(3)]
                    cur = dict(t0=t0, par2=par2, qtu2=qtu2, ksb=ksb, kt2=kt2)

                attn_units = []
                if prev is not None:
                    pv_t0, pv_par2 = prev["t0"], prev["par2"]
                    pv_qtu2, pv_ksb, pv_kt2 = prev["qtu2"], prev["ksb"], prev["kt2"]
                    xot2 = [sbp.tile([128, BT], F32R, tag=f"xot{p}", name=f"xot{p}", bufs=2)
                            for p in range(NPAIR)]
                    pend = []

                    def mk_st(i, kt2=pv_kt2, qtu2=pv_qtu2):
                        def u():
                            emit_st(HEAD_ORDER[i], i % 4, kt2, qtu2)
                        return u

                    def mk_head(i, qtu2=pv_qtu2, ksb=pv_ksb, kt2=pv_kt2,
                                par2=pv_par2, xot2=xot2, pend=pend):
                        def u():
                            h = HEAD_ORDER[i]
                            po = emit_out(h, i % 4, qtu2, par2)
                            if i + 4 < 8:
                                emit_st(HEAD_ORDER[i + 4], i % 4, kt2, qtu2)
                            dv, po_sb = emit_recip(h, po)
                            pend.append((h, dv, po_sb))
                            if len(pend) > 1:
                                emit_tail(*pend.pop(0), xot2)
                            emit_cupd(h, ksb, par2)
                        return u

                    def mk_last_tail(pend=pend, xot2=xot2):
                        def u():
                            emit_tail(*pend.pop(0), xot2)
                        return u

                    def mk_y(t4, fc, xot2=xot2, t0=pv_t0):
                        def u():
                            py = ps.tile([128, GD], F32, tag="proj", name="py", bufs=2)
                            for p in range(NPAIR):
                                nc.tensor.matmul(
                                    py[:],
                                    xot2[p][:, 128 * t4:128 * (t4 + 1)],
                                    wo_sb[p][:, GD * fc:GD * (fc + 1)],
                                    start=(p == 0), stop=(p == NPAIR - 1))
                            ysb = sbp.tile([128, GD], F32, tag="ysb", name="ysb", bufs=3)
                            flex_copy(ysb[:], py[:], GD)
                            nc.scalar.dma_start(
                                y[t0 + 128 * t4:t0 + 128 * (t4 + 1),
                                  GD * fc:GD * (fc + 1)], ysb[:])
                        return u

                    attn_units = ([mk_st(i) for i in range(4)]
                                  + [mk_head(i) for i in range(8)]
                                  + [mk_last_tail()]
                                  + [mk_y(t4, fc) for t4 in range(4) for fc in range(2)])

                # interleave: attention first (its deps are already met),
                # weaving projection units in proportionally
                na, np_ = len(attn_units), len(proj_units)
                if na == 0:
                    for u in proj_units:
                        u()
                else:
                    pi = 0
                    for k, u in enumerate(attn_units):
                        u()
                        want = (k + 1) * np_ // na
                        while pi < want:
                            proj_units[pi]()
                            pi += 1
                    while pi < np_:
                        proj_units[pi]()
                        pi += 1

                prev = cur if ct < NBLK else None

    nc.compile()
    return nc


def _get_nc():
    if "nc" not in _CACHE:
        _CACHE["nc"] = _build()
    return _CACHE["nc"]


def kernel(x, W_qkv, W_out):
    x = np.asarray(x, dtype=np.float32)
    W_qkv = np.asarray(W_qkv, dtype=np.float32)
    W_out = np.asarray(W_out, dtype=np.float32)
    nc = _get_nc()

    xTs = [np.ascontiguousarray(x[b].T) for b in range(B)]
    in_maps = []
    for c in range(NC_CORES):
        b, hg = c // 2, c % 2
        s = slice(hg * GD, (hg + 1) * GD)
        in_maps.append({
            "xT": xTs[b],
            "wqT": np.ascontiguousarray(W_qkv[0 * D:1 * D][s].T),
            "wkT": np.ascontiguousarray(W_qkv[1 * D:2 * D][s].T),
            "wvT": np.ascontiguousarray(W_qkv[2 * D:3 * D][s].T),
            "woT": np.ascontiguousarray(W_out[:, s].T),
        })
    res = run_bass_kernel_spmd(nc, in_maps, core_ids=list(range(NC_CORES)))
    out = np.empty((B, T, D), dtype=np.float32)
    for b in range(B):
        out[b] = res.results[2 * b]["y"] + res.results[2 * b + 1]["y"]
    return out
